# revision 1
# baseline (speedup 1.0000x reference)
"""DivergenceFreeMLP forward kernel for 8 Trainium2 NeuronCores.

Math (per sample z = x.reshape(D), D = 2048):
    a1 = z@W1 + b1;  m1 = a1>0;  h1 = relu(a1)
    a2 = h1@W2 + b2; m2 = a2>0
    s  = z@W1  (= a1 - b1);   r = z@Wo.T
    t1 = s*m1;  u2 = r*m2
    c  = t1@W2;  d = u2@W2.T
    out = sum_j (m2*wop)_j c_j - sum_i (m1*w1p)_i d_i + bp
with wop = Wo@Wp [128], w1p = W1.T@Wp [256].  Algebraically equal to the
reference's jvp/vjp construction:  ((J - J^T) z) @ Wp + bp.

Precision: the output is discontinuous in a1/a2 (ReLU masks gate O(1) terms),
so the mask path (s and a2) uses 3-pass split matmuls with fp16 hi parts and
bf16 lo parts (representation error ~2^-20 -> fp32-equivalent mask decisions;
the PE multiplies 16-bit operands exactly and accumulates fp32, and mixed
fp16 x bf16 operand dtypes are supported).  Value-only paths (r, c, d) are
single-pass fp16; the final feature reduction folds wop/-w1p into fp16
stationary vectors of PE matmuls.  Net accuracy: max-rel ~4e-3 vs fp64
(mask-flip dominated, better than a plain fp32 implementation's ~1.6e-2),
p99.9 ~4e-4 relative.

Sharding: pure data parallelism, batch 8192 -> 1024 per core, weights
replicated.  Device layout is [feature, batch]; the host pre-transposes x and
pre-splits hi/lo bf16 parts.
"""

import numpy as np
import ml_dtypes

import jax
from jax.sharding import Mesh, PartitionSpec
from jax.experimental.shard_map import shard_map

import concourse.tile as tile
import concourse.mybir as mybir
from concourse import bacc
from concourse.bass2jax import _bass_exec_p, install_neuronx_cc_hook, partition_id_tensor

P = 128
D = 2048
KC = D // P          # 16 contraction chunks
GRP = 4              # chunks per DMA group
BC = 1024            # per-core batch
NT = 512             # moving tile (one fp32 PSUM bank)
NTILES = BC // NT    # 2
N_CORES = 8

F32 = mybir.dt.float32
BF16 = mybir.dt.bfloat16
F16 = mybir.dt.float16
AF = mybir.ActivationFunctionType
OP = mybir.AluOpType


def build_nc(iters: int = 1):
    """Per-core Bass program. iters>1 wraps the compute body in a dynamic
    loop (used only for wall-clock-slope HW timing)."""
    nc = bacc.Bacc("TRN2", target_bir_lowering=False, debug=False)

    din = {}
    for name, shape, dt in [
        ("xt_hi", [D, BC], F16),
        ("xt_lo", [D, BC], BF16),
        ("wcat_hi", [D, 384], F16),     # [W1_hi | WoT_hi] fp16
        ("w1_lo", [D, 256], BF16),      # bf16(W1 - fp16(W1))
        # w2p16 cols: [w2h c0 | w2h c1 | w2t_hi(256) | wop(1) | -w1p(2)]
        ("w2p16", [P, 515], F16),
        # w2lo cols: [w2l c0 | w2l c1]  (bf16 residual of fp16(W2))
        ("w2lo", [P, 256], BF16),
        # cpack cols: [b1t(2) | b2t(1) | bp at row0 col3]
        ("cpack", [P, 4], F32),
    ]:
        din[name] = nc.dram_tensor(name, shape, dt, kind="ExternalInput").ap()
    out_dram = nc.dram_tensor("out", [1, BC], F32, kind="ExternalOutput").ap()

    with tile.TileContext(nc) as tc:
        with (
            tc.tile_pool(name="consts", bufs=1) as consts,
            tc.tile_pool(name="xbuf", bufs=1) as xbuf,
            tc.tile_pool(name="work", bufs=2) as work,
            tc.tile_pool(name="psum", bufs=1, space="PSUM") as psum,
        ):
            wcat_sb = consts.tile([P, KC, 384], F16)
            w1lo_sb = consts.tile([P, KC, 256], BF16)
            w2p_sb = consts.tile([P, 515], F16)
            w2lo_sb = consts.tile([P, 256], BF16)
            cp_sb = consts.tile([P, 4], F32)
            xh_sb = xbuf.tile([P, KC, BC], F16)
            xl_sb = xbuf.tile([P, KC, BC], BF16)
            out_sb = xbuf.tile([1, BC], F32)

            wcat_r = din["wcat_hi"].rearrange("(c p) m -> p c m", p=P)
            w1lo_r = din["w1_lo"].rearrange("(c p) m -> p c m", p=P)
            xth = din["xt_hi"].rearrange("(c p) n -> p c n", p=P)
            xtl = din["xt_lo"].rearrange("(c p) n -> p c n", p=P)

            # slice helpers into packed const tiles
            w2h = lambda k: w2p_sb[:, k * P:(k + 1) * P]
            w2l = lambda k: w2lo_sb[:, k * P:(k + 1) * P]
            w2th = lambda m: w2p_sb[:, 256 + m * P:256 + (m + 1) * P]
            wop_bf = w2p_sb[:, 512:513]
            nw1p_bf = lambda m: w2p_sb[:, 513 + m:514 + m]
            b1s = lambda m: cp_sb[:, m:m + 1]
            b2s = cp_sb[:, 2:3]
            bps = cp_sb[0:1, 3:4]


            def load_x(with_weights: bool):
                # need-order: per chunk-group xh, (weights), xl.  The first
                # group's xh/wcat are split per-chunk so MM 0 starts early.
                for g in range(0, KC, GRP):
                    gs = slice(g, g + GRP)
                    if g == 0 and with_weights:
                        nc.sync.dma_start(out=xh_sb[:, 0:1, :], in_=xth[:, 0:1, :])
                        nc.sync.dma_start(out=wcat_sb[:, 0:1, :], in_=wcat_r[:, 0:1, :])
                        nc.sync.dma_start(out=xh_sb[:, 1:GRP, :], in_=xth[:, 1:GRP, :])
                        nc.sync.dma_start(out=wcat_sb[:, 1:GRP, :], in_=wcat_r[:, 1:GRP, :])
                    else:
                        nc.sync.dma_start(out=xh_sb[:, gs, :], in_=xth[:, gs, :])
                        if with_weights:
                            nc.sync.dma_start(out=wcat_sb[:, gs, :], in_=wcat_r[:, gs, :])
                    if with_weights:
                        nc.sync.dma_start(out=w1lo_sb[:, gs, :], in_=w1lo_r[:, gs, :])
                # xl after all hi-parts/weights: ntile 0 defers its xl passes
                for g in range(0, KC, GRP):
                    gs = slice(g, g + GRP)
                    nc.sync.dma_start(out=xl_sb[:, gs, :], in_=xtl[:, gs, :])
                if with_weights:
                    nc.sync.dma_start(out=w2p_sb, in_=din["w2p16"])
                    nc.sync.dma_start(out=w2lo_sb, in_=din["w2lo"])
                    nc.sync.dma_start(out=cp_sb, in_=din["cpack"])

            def load_weights():
                for g in range(0, KC, GRP):
                    gs = slice(g, g + GRP)
                    nc.sync.dma_start(out=wcat_sb[:, gs, :], in_=wcat_r[:, gs, :])
                    nc.sync.dma_start(out=w1lo_sb[:, gs, :], in_=w1lo_r[:, gs, :])
                nc.sync.dma_start(out=w2p_sb, in_=din["w2p16"])
                nc.sync.dma_start(out=w2lo_sb, in_=din["w2lo"])
                nc.sync.dma_start(out=cp_sb, in_=din["cpack"])

            def body(load: bool):
                if load:
                    load_x(with_weights=False)
                # both ntiles' big matmuls in one interleaved stream
                ps_s = {}; ps_r = {}
                cnt = {}
                for nt in range(NTILES):
                    for m in range(2):
                        ps_s[(nt, m)] = psum.tile([P, NT], F32, tag=f"t{nt*3+m}", name=f"ps_s{nt}{m}")
                    ps_r[nt] = psum.tile([P, NT], F32, tag=f"t{nt*3+2}", name=f"ps_r{nt}")
                    cnt[(nt,0)] = 0; cnt[(nt,1)] = 0; cnt[(nt,'r')] = 0
                n_s, n_r = 3 * KC, KC

                def s_hi_group(nt, m, g):
                    ns = slice(nt * NT, (nt + 1) * NT)
                    mcols = slice(m * P, (m + 1) * P)
                    for k in range(g, g + GRP):
                        nc.tensor.matmul(ps_s[(nt,m)], wcat_sb[:, k, mcols], xh_sb[:, k, ns],
                                         start=(cnt[(nt,m)] == 0), stop=(cnt[(nt,m)] == n_s - 1)); cnt[(nt,m)] += 1
                    for k in range(g, g + GRP):
                        nc.tensor.matmul(ps_s[(nt,m)], w1lo_sb[:, k, mcols], xh_sb[:, k, ns],
                                         start=False, stop=(cnt[(nt,m)] == n_s - 1)); cnt[(nt,m)] += 1

                def s_lo_group(nt, m, g):
                    ns = slice(nt * NT, (nt + 1) * NT)
                    mcols = slice(m * P, (m + 1) * P)
                    for k in range(g, g + GRP):
                        nc.tensor.matmul(ps_s[(nt,m)], wcat_sb[:, k, mcols], xl_sb[:, k, ns],
                                         start=False, stop=(cnt[(nt,m)] == n_s - 1)); cnt[(nt,m)] += 1

                def r_group(nt, g):
                    ns = slice(nt * NT, (nt + 1) * NT)
                    for k in range(g, g + GRP):
                        nc.tensor.matmul(ps_r[nt], wcat_sb[:, k, 256:384], xh_sb[:, k, ns],
                                         start=(cnt[(nt,'r')] == 0), stop=(cnt[(nt,'r')] == n_r - 1)); cnt[(nt,'r')] += 1

                # hi sweep over both ntiles, then lo sweeps, ntile0 first
                for g in range(0, KC, GRP):
                    for nt in range(NTILES):
                        s_hi_group(nt, 0, g); s_hi_group(nt, 1, g); r_group(nt, g)
                for g in range(0, KC, GRP):
                    s_lo_group(0, 0, g); s_lo_group(0, 1, g)
                for g in range(0, KC, GRP):
                    s_lo_group(1, 0, g); s_lo_group(1, 1, g)

                # interleaved epilogues: both mask chains first (DVE/ACT of
                # chain-1 hides under epilogue-0's PE matmuls), then the two
                # value/final sections.
                m1 = {}; t1b = {}; h1h = {}; h1f = {}; h1l = {}
                for nt in range(NTILES):
                    pss = [ps_s[(nt, 0)], ps_s[(nt, 1)]]
                    m1[nt] = work.tile([P, 2, NT], F32, tag=f"m1_{nt}", name=f"m1_{nt}")
                    t1b[nt] = work.tile([P, 2, NT], F16, tag=f"t1b_{nt}", name=f"t1b_{nt}")
                    h1h[nt] = work.tile([P, 2, NT], F16, tag=f"h1h_{nt}", name=f"h1h_{nt}")
                    h1f[nt] = work.tile([P, 2, NT], F32, tag=f"h1f_{nt}", name=f"h1f_{nt}")
                    h1l[nt] = work.tile([P, 2, NT], BF16, tag=f"h1l_{nt}", name=f"h1l_{nt}")
                    for m in range(2):
                        nc.vector.tensor_scalar(m1[nt][:, m], pss[m], b1s(m), 0.0, OP.add, OP.is_gt)
                        nc.scalar.activation(h1h[nt][:, m], pss[m], AF.Relu, bias=b1s(m))
                        nc.scalar.activation(h1f[nt][:, m], pss[m], AF.Relu, bias=b1s(m))
                    for m in range(2):
                        nc.vector.tensor_tensor(h1l[nt][:, m], h1f[nt][:, m], h1h[nt][:, m], OP.subtract)

                f_tag = {0: "t2", 1: "t5"}
                for nt in range(NTILES):
                    ns = slice(nt * NT, (nt + 1) * NT)
                    pss = [ps_s[(nt, 0)], ps_s[(nt, 1)]]
                    psr = ps_r[nt]
                    ps_a2 = psum.tile([P, NT], F32, tag="t6", name=f"ps_a2_{nt}")
                    a2_seq = [(w2h(0), h1h[nt][:, 0]), (w2h(1), h1h[nt][:, 1]),
                              (w2l(0), h1h[nt][:, 0]), (w2l(1), h1h[nt][:, 1]),
                              (w2h(0), h1l[nt][:, 0]), (w2h(1), h1l[nt][:, 1])]
                    for i, (wt, ht) in enumerate(a2_seq):
                        nc.tensor.matmul(ps_a2, wt, ht,
                                         start=(i == 0), stop=(i == len(a2_seq) - 1))
                    m2 = work.tile([P, NT], F32, tag="m2")
                    nc.vector.tensor_scalar(m2, ps_a2, b2s, 0.0, OP.add, OP.is_gt)
                    u2b = work.tile([P, NT], F16, tag="u2b")
                    nc.vector.tensor_tensor(u2b, psr, m2, OP.mult)
                    for m in range(2):
                        nc.vector.tensor_tensor(t1b[nt][:, m], pss[m], m1[nt][:, m], OP.mult)
                    ps_c = psum.tile([P, NT], F32, tag="t7", name=f"ps_c_{nt}")
                    for k in range(2):
                        nc.tensor.matmul(ps_c, w2h(k), t1b[nt][:, k], start=(k == 0), stop=(k == 1))
                    ps_d = [psum.tile([P, NT], F32, tag=f"t{nt*3+m}", name=f"ps_d_{nt}{m}")
                            for m in range(2)]
                    for m in range(2):
                        nc.tensor.matmul(ps_d[m], w2th(m), u2b, start=True, stop=True)
                    e1 = work.tile([P, NT], F16, tag="e1")
                    nc.vector.tensor_tensor(e1, ps_c, m2, OP.mult)
                    e2 = work.tile([P, 2, NT], F16, tag="e2")
                    for m in range(2):
                        nc.vector.tensor_tensor(e2[:, m], ps_d[m], m1[nt][:, m], OP.mult)
                    ps_f = psum.tile([1, NT], F32, tag=f_tag[nt], name=f"ps_f_{nt}")
                    nc.tensor.matmul(ps_f, wop_bf, e1, start=True, stop=False)
                    nc.tensor.matmul(ps_f, nw1p_bf(0), e2[:, 0], start=False, stop=False)
                    nc.tensor.matmul(ps_f, nw1p_bf(1), e2[:, 1], start=False, stop=True)
                    nc.scalar.activation(out_sb[:, ns], ps_f, AF.Identity, bias=bps)
                nc.sync.dma_start(out=out_dram, in_=out_sb)

            if iters == 1:
                load_x(with_weights=True)
                body(load=False)
            else:
                load_weights()
                with tc.For_i(0, iters, 1):
                    body(load=True)

    nc.compile()
    return nc


class _Runner:
    """Minimal PJRT SPMD runner (axon path), keeps the jitted callable."""

    def __init__(self, nc, n_cores: int):
        install_neuronx_cc_hook()
        self.n_cores = n_cores
        pid_name = nc.partition_id_tensor.name if nc.partition_id_tensor else None
        in_names, out_names, out_avals, zero_outs = [], [], [], []
        for alloc in nc.m.functions[0].allocations:
            if not isinstance(alloc, mybir.MemoryLocationSet):
                continue
            name = alloc.memorylocations[0].name
            if alloc.kind == "ExternalInput":
                if name != pid_name:
                    in_names.append(name)
            elif alloc.kind == "ExternalOutput":
                out_names.append(name)
                shape = tuple(alloc.tensor_shape)
                dtype = mybir.dt.np(alloc.dtype)
                out_avals.append(jax.core.ShapedArray(shape, dtype))
                zero_outs.append(np.zeros(shape, dtype))
        self.in_names, self.out_names = in_names, out_names
        self.out_avals, self.zero_outs = out_avals, zero_outs
        n_params, n_outs = len(in_names), len(out_avals)
        all_in_names = in_names + out_names + ([pid_name] if pid_name else [])

        def _body(*args):
            operands = list(args)
            if pid_name is not None:
                operands.append(partition_id_tensor())
            return tuple(_bass_exec_p.bind(
                *operands,
                out_avals=tuple(out_avals),
                in_names=tuple(all_in_names),
                out_names=tuple(out_names),
                lowering_input_output_aliases=(),
                sim_require_finite=True,
                sim_require_nnan=True,
                nc=nc,
            ))

        devices = jax.devices()[:n_cores]
        mesh = Mesh(np.asarray(devices), ("core",))
        self.sharded = jax.jit(
            shard_map(_body, mesh=mesh,
                      in_specs=(PartitionSpec("core"),) * (n_params + n_outs),
                      out_specs=(PartitionSpec("core"),) * n_outs,
                      check_rep=False),
            donate_argnums=tuple(range(n_params, n_params + n_outs)),
            keep_unused=True,
        )

    def __call__(self, in_maps):
        concat_in = [
            np.concatenate([np.asarray(in_maps[c][name]) for c in range(self.n_cores)], axis=0)
            for name in self.in_names
        ]
        concat_zeros = [
            np.zeros((self.n_cores * z.shape[0], *z.shape[1:]), z.dtype)
            for z in self.zero_outs
        ]
        out_arrs = self.sharded(*concat_in, *concat_zeros)
        jax.block_until_ready(out_arrs)
        return [
            {name: np.asarray(out_arrs[i]).reshape(self.n_cores, *self.out_avals[i].shape)[c]
             for i, name in enumerate(self.out_names)}
            for c in range(self.n_cores)
        ]


_CACHE = {}


def _get_runner(iters: int = 1):
    if iters not in _CACHE:
        _CACHE[iters] = _Runner(build_nc(iters), N_CORES)
    return _CACHE[iters]


def _bf(a):
    return np.asarray(a, np.float32).astype(ml_dtypes.bfloat16)


def prepare_inputs(x, W1, b1, W2, b2, Wo, bo, Wp, bp):
    """Host prep: transpose + bf16 hi/lo splits + weight folds -> per-core maps."""
    x = np.asarray(x, np.float32)
    W1 = np.asarray(W1, np.float32); b1 = np.asarray(b1, np.float32)
    W2 = np.asarray(W2, np.float32); b2 = np.asarray(b2, np.float32)
    Wo = np.asarray(Wo, np.float32); Wp = np.asarray(Wp, np.float32)
    bp = np.asarray(bp, np.float32)
    B = x.shape[0]
    assert B == BC * N_CORES, f"expected batch {BC * N_CORES}, got {B}"
    X = x.reshape(B, D)

    XT = np.ascontiguousarray(X.T)
    xt_hi = XT.astype(np.float16)
    xt_lo = (XT - xt_hi.astype(np.float32)).astype(ml_dtypes.bfloat16)

    wcat = np.concatenate([W1, Wo.T], axis=1)            # [D, 384]
    wcat_hi = wcat.astype(np.float16)
    w1_lo = (W1 - wcat_hi[:, :256].astype(np.float32)).astype(ml_dtypes.bfloat16)
    w2_hi = W2.astype(np.float16)                        # [256, 128]
    w2_lo = (W2 - w2_hi.astype(np.float32)).astype(ml_dtypes.bfloat16)
    w2t_hi = np.ascontiguousarray(W2.T).astype(np.float16)  # [128, 256]
    wop = (Wo @ Wp).astype(np.float32).reshape(P, 1)
    w1p = (W1.T @ Wp).astype(np.float32).reshape(2, P).T  # [128, 2]
    # w2p16: [w2h c0 | w2h c1 | w2t_hi | wop | -w1p] (fp16)
    w2p16 = np.concatenate(
        [w2_hi[:P], w2_hi[P:], w2t_hi,
         wop.astype(np.float16), (-w1p).astype(np.float16)], axis=1)
    w2lo = np.concatenate([w2_lo[:P], w2_lo[P:]], axis=1)  # bf16
    cpack = np.zeros((P, 4), np.float32)
    cpack[:, 0:2] = b1.reshape(2, P).T
    cpack[:, 2:3] = b2.reshape(P, 1)
    cpack[0, 3] = float(bp.reshape(-1)[0])

    shared = {
        "wcat_hi": wcat_hi,
        "w1_lo": w1_lo,
        "w2p16": np.ascontiguousarray(w2p16),
        "w2lo": np.ascontiguousarray(w2lo),
        "cpack": cpack,
    }
    in_maps = []
    for c in range(N_CORES):
        cs = slice(c * BC, (c + 1) * BC)
        m = dict(shared)
        m["xt_hi"] = np.ascontiguousarray(xt_hi[:, cs])
        m["xt_lo"] = np.ascontiguousarray(xt_lo[:, cs])
        in_maps.append(m)
    return in_maps


def kernel(**inputs):
    in_maps = prepare_inputs(**inputs)
    runner = _get_runner(1)
    results = runner(in_maps)
    out = np.concatenate([results[c]["out"][0] for c in range(N_CORES)])
    return out.reshape(BC * N_CORES, 1).astype(np.float32)



# revision 2
# speedup vs baseline: 1.0308x; 1.0308x over previous
"""DivergenceFreeMLP forward kernel for 8 Trainium2 NeuronCores (v2).

Math (per sample z = x.reshape(D), D = 2048):
    a1 = z@W1 + b1;  m1 = a1>0
    s  = z@W1;   r = z@Wo.T
    a2 = relu(a1)@W2 + b2;  m2 = a2>0
    out = sum_j m2_j * [ (t1@W2')_j - r_j * (W2^T (m1*w1p))_j ] + bp
  where t1 = s*m1, W2' = W2 * wop[None, :] (wop = Wo@Wp folded host-side),
  w1p = W1.T@Wp.  Algebraically equal to ((J - J^T) z) @ Wp + bp.

Precision: identical mask scheme to v1 (3-pass fp16/bf16 split for s, 6-mm
split chain for a2) -- mask flips cost ~2e-2 relative each, so masks must be
fp32-grade.  Value paths fp16 single-pass.

v2 changes vs v1 (same math, less PE/tail time):
  - d-path replaced by q-path + host-folded wop: final reduce is ONE
    ones-vector matmul per epilogue instead of three.
  - epilogue split into 4 instances of 256 cols, interleaved into the big
    matmul stream so the serial tail after the last big matmul shrinks.
"""

import numpy as np
import ml_dtypes

import jax
from jax.sharding import Mesh, PartitionSpec
from jax.experimental.shard_map import shard_map

import concourse.tile as tile
import concourse.mybir as mybir
from concourse import bacc
from concourse.bass2jax import _bass_exec_p, install_neuronx_cc_hook, partition_id_tensor

P = 128
D = 2048
KC = D // P          # 16 contraction chunks
GRP = 4              # chunks per DMA group
BC = 1024            # per-core batch
NT = 512             # moving tile (one fp32 PSUM bank)
NTILES = BC // NT    # 2
NE = 256             # epilogue instance width
NEP = BC // NE       # 4 epilogue instances
N_CORES = 8

F32 = mybir.dt.float32
BF16 = mybir.dt.bfloat16
F16 = mybir.dt.float16
AF = mybir.ActivationFunctionType
OP = mybir.AluOpType


def build_nc(iters: int = 1):
    """Per-core Bass program. iters>1 wraps the compute body in a dynamic
    loop (used only for wall-clock-slope HW timing)."""
    nc = bacc.Bacc("TRN2", target_bir_lowering=False, debug=False)

    din = {}
    for name, shape, dt in [
        ("xt_hi", [D, BC], F16),
        ("xt_lo", [D, BC], BF16),
        ("wcat_hi", [D, 384], F16),     # [W1_hi | WoT_hi] fp16
        ("w1_lo", [D, 256], BF16),      # bf16(W1 - fp16(W1))
        # w2p16 cols: [w2h(256) | w2nb(256) | w2q(256) | ones]  fp16
        # (w2nb = -W2*b1-rows; w2q = W2*w1p-rows*16; ones = reduce vector)
        ("w2p16", [P, 769], F16),
        # w2lo cols: [w2l c0 | w2l c1]  (bf16 residual of fp16(W2))
        ("w2lo", [P, 256], BF16),
        # cpack cols: [b1t(2) | b2t(1) | bp at row0 col3 | wop(1)]
        ("cpack", [P, 5], F32),
    ]:
        din[name] = nc.dram_tensor(name, shape, dt, kind="ExternalInput").ap()
    out_dram = nc.dram_tensor("out", [1, BC], F32, kind="ExternalOutput").ap()

    with tile.TileContext(nc) as tc:
        with (
            tc.tile_pool(name="consts", bufs=1) as consts,
            tc.tile_pool(name="xbuf", bufs=1) as xbuf,
            tc.tile_pool(name="work", bufs=2) as work,
            tc.tile_pool(name="psum", bufs=1, space="PSUM") as psum,
        ):
            wcat_sb = consts.tile([P, KC, 384], F16)
            w1lo_sb = consts.tile([P, KC, 256], BF16)
            w2p_sb = consts.tile([P, 769], F16)
            w2lo_sb = consts.tile([P, 256], BF16)
            cp_sb = consts.tile([P, 5], F32)
            xh_sb = xbuf.tile([P, KC, BC], F16)
            xl_sb = xbuf.tile([P, KC, BC], BF16)
            out_sb = xbuf.tile([1, BC], F32)

            wcat_r = din["wcat_hi"].rearrange("(c p) m -> p c m", p=P)
            w1lo_r = din["w1_lo"].rearrange("(c p) m -> p c m", p=P)
            xth = din["xt_hi"].rearrange("(c p) n -> p c n", p=P)
            xtl = din["xt_lo"].rearrange("(c p) n -> p c n", p=P)

            # slice helpers into packed const tiles
            w2h = lambda k: w2p_sb[:, k * P:(k + 1) * P]               # W2
            w2nb = lambda k: w2p_sb[:, 256 + k * P:256 + (k + 1) * P]  # -W2*b1
            w2q = lambda k: w2p_sb[:, 512 + k * P:512 + (k + 1) * P]   # W2*w1p*16
            ones16 = w2p_sb[:, 768:769]
            w2l = lambda k: w2lo_sb[:, k * P:(k + 1) * P]
            b1s = lambda m: cp_sb[:, m:m + 1]
            b2s = cp_sb[:, 2:3]
            bps = cp_sb[0:1, 3:4]
            wops = cp_sb[:, 4:5]

            def load_x(with_weights: bool):
                # Three parallel DMA queues: xh on the SP HWDGE queue, weights
                # on the ACT HWDGE queue, xl on the GPSIMD SWDGE queue.  The
                # first group's xh is split per-chunk so MM 0 starts early.
                for g in range(0, KC, GRP):
                    gs = slice(g, g + GRP)
                    if g == 0:
                        nc.sync.dma_start(out=xh_sb[:, 0:1, :], in_=xth[:, 0:1, :])
                        nc.sync.dma_start(out=xh_sb[:, 1:GRP, :], in_=xth[:, 1:GRP, :])
                    else:
                        nc.sync.dma_start(out=xh_sb[:, gs, :], in_=xth[:, gs, :])
                    if with_weights:
                        nc.scalar.dma_start(out=wcat_sb[:, gs, :], in_=wcat_r[:, gs, :])
                        nc.scalar.dma_start(out=w1lo_sb[:, gs, :], in_=w1lo_r[:, gs, :])
                for g in range(0, KC, GRP):
                    gs = slice(g, g + GRP)
                    nc.sync.dma_start(out=xl_sb[:, gs, :], in_=xtl[:, gs, :])
                if with_weights:
                    nc.scalar.dma_start(out=w2p_sb, in_=din["w2p16"])
                    nc.scalar.dma_start(out=w2lo_sb, in_=din["w2lo"])
                    nc.scalar.dma_start(out=cp_sb, in_=din["cpack"])

            def load_weights():
                for g in range(0, KC, GRP):
                    gs = slice(g, g + GRP)
                    nc.scalar.dma_start(out=wcat_sb[:, gs, :], in_=wcat_r[:, gs, :])
                    nc.scalar.dma_start(out=w1lo_sb[:, gs, :], in_=w1lo_r[:, gs, :])
                nc.scalar.dma_start(out=w2p_sb, in_=din["w2p16"])
                nc.scalar.dma_start(out=w2lo_sb, in_=din["w2lo"])
                nc.scalar.dma_start(out=cp_sb, in_=din["cpack"])

            def body(load: bool):
                if load:
                    load_x(with_weights=False)
                # ---- big matmul stream: 3-pass s + 1-pass r per ntile ----
                ps_s = {}; ps_r = {}; cnt = {}
                for nt in range(NTILES):
                    for m in range(2):
                        ps_s[(nt, m)] = psum.tile([P, NT], F32, tag=f"t{nt*3+m}",
                                                  name=f"ps_s{nt}{m}")
                    ps_r[nt] = psum.tile([P, NT], F32, tag=f"t{nt*3+2}", name=f"ps_r{nt}")
                    cnt[(nt, 0)] = 0; cnt[(nt, 1)] = 0; cnt[(nt, 'r')] = 0
                n_s, n_r = 3 * KC, KC

                def s_hi_group(nt, m, g):
                    ns = slice(nt * NT, (nt + 1) * NT)
                    mcols = slice(m * P, (m + 1) * P)
                    for k in range(g, g + GRP):
                        nc.tensor.matmul(ps_s[(nt, m)], wcat_sb[:, k, mcols], xh_sb[:, k, ns],
                                         start=(cnt[(nt, m)] == 0),
                                         stop=(cnt[(nt, m)] == n_s - 1)); cnt[(nt, m)] += 1

                def s_wl_group(nt, m, g):
                    ns = slice(nt * NT, (nt + 1) * NT)
                    mcols = slice(m * P, (m + 1) * P)
                    for k in range(g, g + GRP):
                        nc.tensor.matmul(ps_s[(nt, m)], w1lo_sb[:, k, mcols], xh_sb[:, k, ns],
                                         start=False,
                                         stop=(cnt[(nt, m)] == n_s - 1)); cnt[(nt, m)] += 1

                def s_lo_group(nt, m, g):
                    ns = slice(nt * NT, (nt + 1) * NT)
                    mcols = slice(m * P, (m + 1) * P)
                    for k in range(g, g + GRP):
                        nc.tensor.matmul(ps_s[(nt, m)], wcat_sb[:, k, mcols], xl_sb[:, k, ns],
                                         start=False,
                                         stop=(cnt[(nt, m)] == n_s - 1)); cnt[(nt, m)] += 1

                def r_group(nt, g):
                    ns = slice(nt * NT, (nt + 1) * NT)
                    for k in range(g, g + GRP):
                        nc.tensor.matmul(ps_r[nt], wcat_sb[:, k, 256:384], xh_sb[:, k, ns],
                                         start=(cnt[(nt, 'r')] == 0),
                                         stop=(cnt[(nt, 'r')] == n_r - 1)); cnt[(nt, 'r')] += 1

                # ---- epilogue instances: i-th covers cols [i*NE, (i+1)*NE) ----
                # instance i reads ntile nt=i//2, psum col slice es(i).
                def es(i):
                    base = (i % 2) * NE
                    return slice(base, base + NE)

                # bank (tag) map per instance, chosen so PE never waits on a
                # WAR bank hazard (see scheduling notes).
                A2T = ["t6", "t6", "t6", "t3"]
                CT = ["t7", "t0", "t7", "t4"]
                QT = ["t1", "t1", "t2", "t0"]
                FT = ["t6", "t6", "t2", "t5"]

                m1_t = {}; t1b_t = {}; mq_t = {}; h1h_t = {}; h1f_t = {}; h1l_t = {}
                m2_t = {}; rq_t = {}; tt_t = {}; e_t = {}; qs_t = {}
                ps_a2 = {}; ps_c = {}; ps_q = {}; ps_f = {}

                def ep_lead_dve(i):
                    """fp16 {0,1} mask m1 for instance i (DVE) -- serves as
                    the MOVING operand of the b1/w1p-folded matmuls."""
                    nt = i // 2
                    sl = es(i)
                    pss = [ps_s[(nt, 0)][:, sl], ps_s[(nt, 1)][:, sl]]
                    m1_t[i] = work.tile([P, 2, NE], F16, tag=f"m1_{i%2}", name=f"m1_{i}")
                    for m in range(2):
                        nc.vector.tensor_scalar(m1_t[i][:, m], pss[m], b1s(m), 0.0,
                                                OP.add, OP.is_gt)

                def ep_lead_gp(i):
                    """h1 fp16-residual (Pool/GPSIMD)."""
                    h1l_t[i] = work.tile([P, 2, NE], BF16, tag=f"h1l_{i%2}", name=f"h1l_{i}")
                    for m in range(2):
                        nc.gpsimd.tensor_tensor(h1l_t[i][:, m], h1f_t[i][:, m],
                                                h1h_t[i][:, m], OP.subtract)

                def ep_lead_act(i):
                    """relu hi part for the a2 chain (ACT engine); the fp32
                    relu runs on DVE (fused add+max) to shorten the ACT
                    critical path into a2 of the last instances."""
                    nt = i // 2
                    sl = es(i)
                    pss = [ps_s[(nt, 0)][:, sl], ps_s[(nt, 1)][:, sl]]
                    h1h_t[i] = work.tile([P, 2, NE], F16, tag=f"h1h_{i%2}", name=f"h1h_{i}")
                    h1f_t[i] = work.tile([P, 2, NE], F32, tag=f"h1f_{i%2}", name=f"h1f_{i}")
                    for m in range(2):
                        nc.scalar.activation(h1h_t[i][:, m], pss[m], AF.Relu, bias=b1s(m))
                    for m in range(2):
                        nc.vector.tensor_scalar(h1f_t[i][:, m], pss[m], b1s(m), 0.0,
                                                OP.add, OP.max)

                def ep_a2c_mm(i):
                    """a2 6-mm mask chain + c 4-mm (PE).
                    c psum = W2^T h1h - (W2*b1)^T m1  ( = W2^T t1 to fp16 )."""
                    ps_a2[i] = psum.tile([P, NE], F32, tag=A2T[i], name=f"ps_a2_{i}")
                    a2_seq = [(w2h(0), h1h_t[i][:, 0]), (w2h(1), h1h_t[i][:, 1]),
                              (w2l(0), h1h_t[i][:, 0]), (w2l(1), h1h_t[i][:, 1]),
                              (w2h(0), h1l_t[i][:, 0]), (w2h(1), h1l_t[i][:, 1])]
                    for j, (wt, ht) in enumerate(a2_seq):
                        nc.tensor.matmul(ps_a2[i], wt, ht,
                                         start=(j == 0), stop=(j == len(a2_seq) - 1))
                    ps_c[i] = psum.tile([P, NE], F32, tag=CT[i], name=f"ps_c_{i}")
                    c_seq = [(w2h(0), h1h_t[i][:, 0]), (w2h(1), h1h_t[i][:, 1]),
                             (w2nb(0), m1_t[i][:, 0]), (w2nb(1), m1_t[i][:, 1])]
                    for j, (wt, ht) in enumerate(c_seq):
                        nc.tensor.matmul(ps_c[i], wt, ht,
                                         start=(j == 0), stop=(j == len(c_seq) - 1))

                def ep_q_mm(i):
                    ps_q[i] = psum.tile([P, NE], F32, tag=QT[i], name=f"ps_q_{i}")
                    for k in range(2):
                        nc.tensor.matmul(ps_q[i], w2q(k), m1_t[i][:, k],
                                         start=(k == 0), stop=(k == 1))

                def ep_m2(i):
                    m2_t[i] = work.tile([P, NE], F32, tag=f"m2_{i%2}", name=f"m2_{i}")
                    nc.vector.tensor_scalar(m2_t[i], ps_a2[i], b2s, 0.0, OP.add, OP.is_gt)

                def ep_qcopy(i):
                    """q -> SBUF on ACT, undoing the x16 stationary scale
                    (two-PSUM-input tensor_tensor is illegal, hence a copy)."""
                    qs_t[i] = work.tile([P, NE], F32, tag=f"qs_{i%2}", name=f"q_sb_{i}")
                    nc.scalar.activation(qs_t[i], ps_q[i], AF.Identity, scale=1.0 / 16.0)

                def ep_combine(i):
                    """rq = r*q;  T = c*wop - rq  (DVE, wop per-partition)."""
                    nt = i // 2
                    sl = es(i)
                    rq_t[i] = work.tile([P, NE], F32, tag=f"rq_{i%2}", name=f"rq_{i}")
                    nc.vector.tensor_tensor(rq_t[i], ps_r[nt][:, sl], qs_t[i], OP.mult)
                    tt_t[i] = work.tile([P, NE], F32, tag=f"tt_{i%2}", name=f"tt_{i}")
                    nc.vector.scalar_tensor_tensor(tt_t[i], ps_c[i], wops, rq_t[i],
                                                   OP.mult, OP.subtract)

                def ep_e(i):
                    """E = T*m2 (instances 0/1 on Pool, 2/3 on DVE -- balances
                    the tail era where Pool still holds h1l work)."""
                    e_t[i] = work.tile([P, NE], F16, tag=f"e_{i%2}", name=f"e_{i}")
                    eng = nc.gpsimd if i < 2 else nc.vector
                    eng.tensor_tensor(e_t[i], tt_t[i], m2_t[i], OP.mult)

                def ep_f_mm(i):
                    ps_f[i] = psum.tile([1, NE], F32, tag=FT[i], name=f"ps_f_{i}")
                    nc.tensor.matmul(ps_f[i], ones16, e_t[i], start=True, stop=True)

                def ep_out(i):
                    nt = i // 2
                    ns = slice(nt * NT + (i % 2) * NE, nt * NT + (i % 2 + 1) * NE)
                    nc.scalar.activation(out_sb[:, ns], ps_f[i], AF.Identity, bias=bps)
                    if i == 1:  # first half on the ACT queue (fully hidden)
                        nc.scalar.dma_start(out=out_dram[:, 0:NT], in_=out_sb[:, 0:NT])
                    elif i == 3:  # last half on the idle SP queue
                        nc.sync.dma_start(out=out_dram[:, NT:BC], in_=out_sb[:, NT:BC])

                # ---- emission (per-engine program order == readiness order) --
                # PE: hi sweep over both ntiles (hi+r first within a group so
                # the w1lo DMA of the group has more time to land)
                for g in range(0, KC, GRP):
                    for nt in range(NTILES):
                        s_hi_group(nt, 0, g); s_hi_group(nt, 1, g); r_group(nt, g)
                    for nt in range(NTILES):
                        s_wl_group(nt, 0, g); s_wl_group(nt, 1, g)
                # PE: nt0 lo sweep -> ps_s(0) complete
                for g in range(0, KC, GRP):
                    s_lo_group(0, 0, g); s_lo_group(0, 1, g)
                # DVE/ACT/GPSIMD: leads for ep0/ep1 (overlap nt1 lo sweep on PE)
                ep_lead_act(0); ep_lead_act(1)
                ep_lead_dve(0)
                ep_lead_gp(0)
                ep_lead_dve(1)
                ep_lead_gp(1)
                # PE: first half of nt1 lo sweep
                s_lo_group(1, 0, 0); s_lo_group(1, 1, 0)
                s_lo_group(1, 0, GRP); s_lo_group(1, 1, GRP)
                # PE: ep0 a2+c
                ep_a2c_mm(0)
                ep_m2(0)
                # PE: second half of nt1 lo sweep -> ps_s(1)/ps_r(1) complete
                s_lo_group(1, 0, 2 * GRP); s_lo_group(1, 1, 2 * GRP)
                s_lo_group(1, 0, 3 * GRP); s_lo_group(1, 1, 3 * GRP)
                # leads for ep2/ep3 run during the ep1/ep2 PE batches
                ep_lead_act(2); ep_lead_act(3)
                ep_a2c_mm(1)
                ep_m2(1)           # before leads: frees t6 for a2_2 promptly
                ep_lead_dve(2)
                ep_lead_gp(2)
                ep_lead_dve(3)
                ep_lead_gp(3)
                ep_q_mm(0)
                ep_qcopy(0)
                ep_combine(0)
                ep_a2c_mm(2)
                ep_m2(2)
                ep_e(0)
                ep_q_mm(1)
                ep_qcopy(1)
                ep_combine(1)
                ep_f_mm(0)
                ep_out(0)
                ep_a2c_mm(3)
                ep_m2(3)
                ep_e(1)
                ep_q_mm(2)
                ep_qcopy(2)
                ep_combine(2)
                ep_f_mm(1)
                ep_out(1)
                # PE tail: ep3 q+f, ep2 f
                ep_q_mm(3)
                ep_qcopy(3)
                ep_combine(3)
                ep_e(2)
                ep_f_mm(2)
                ep_out(2)
                ep_e(3)
                ep_f_mm(3)
                ep_out(3)

            def pe_warmup(n_mm: int):
                """Tiny dummy matmuls that keep PE continuously busy during the
                initial x/weight DMA, so the p-state ramp (full clock only
                after ~3us of sustained PE activity) completes before the real
                matmul stream starts."""
                wz = xbuf.tile([P, 16], F16)
                nc.vector.memset(wz, 0.0)
                ps_w = psum.tile([16, 16], F32, tag="t7", name="ps_warm")
                for j in range(n_mm):
                    nc.tensor.matmul(ps_w, wz, wz, start=(j == 0), stop=(j == n_mm - 1))

            if iters == 1:
                pe_warmup(220)
                load_x(with_weights=True)
                body(load=False)
            else:
                load_weights()
                with tc.For_i(0, iters, 1):
                    body(load=True)

    nc.compile()
    return nc


class _Runner:
    """Minimal PJRT SPMD runner (axon path), keeps the jitted callable."""

    def __init__(self, nc, n_cores: int):
        install_neuronx_cc_hook()
        self.n_cores = n_cores
        pid_name = nc.partition_id_tensor.name if nc.partition_id_tensor else None
        in_names, out_names, out_avals, zero_outs = [], [], [], []
        for alloc in nc.m.functions[0].allocations:
            if not isinstance(alloc, mybir.MemoryLocationSet):
                continue
            name = alloc.memorylocations[0].name
            if alloc.kind == "ExternalInput":
                if name != pid_name:
                    in_names.append(name)
            elif alloc.kind == "ExternalOutput":
                out_names.append(name)
                shape = tuple(alloc.tensor_shape)
                dtype = mybir.dt.np(alloc.dtype)
                out_avals.append(jax.core.ShapedArray(shape, dtype))
                zero_outs.append(np.zeros(shape, dtype))
        self.in_names, self.out_names = in_names, out_names
        self.out_avals, self.zero_outs = out_avals, zero_outs
        n_params, n_outs = len(in_names), len(out_avals)
        all_in_names = in_names + out_names + ([pid_name] if pid_name else [])

        def _body(*args):
            operands = list(args)
            if pid_name is not None:
                operands.append(partition_id_tensor())
            return tuple(_bass_exec_p.bind(
                *operands,
                out_avals=tuple(out_avals),
                in_names=tuple(all_in_names),
                out_names=tuple(out_names),
                lowering_input_output_aliases=(),
                sim_require_finite=True,
                sim_require_nnan=True,
                nc=nc,
            ))

        devices = jax.devices()[:n_cores]
        mesh = Mesh(np.asarray(devices), ("core",))
        self.sharded = jax.jit(
            shard_map(_body, mesh=mesh,
                      in_specs=(PartitionSpec("core"),) * (n_params + n_outs),
                      out_specs=(PartitionSpec("core"),) * n_outs,
                      check_rep=False),
            donate_argnums=tuple(range(n_params, n_params + n_outs)),
            keep_unused=True,
        )

    def __call__(self, in_maps):
        concat_in = [
            np.concatenate([np.asarray(in_maps[c][name]) for c in range(self.n_cores)], axis=0)
            for name in self.in_names
        ]
        concat_zeros = [
            np.zeros((self.n_cores * z.shape[0], *z.shape[1:]), z.dtype)
            for z in self.zero_outs
        ]
        out_arrs = self.sharded(*concat_in, *concat_zeros)
        jax.block_until_ready(out_arrs)
        return [
            {name: np.asarray(out_arrs[i]).reshape(self.n_cores, *self.out_avals[i].shape)[c]
             for i, name in enumerate(self.out_names)}
            for c in range(self.n_cores)
        ]


_CACHE = {}


def _get_runner(iters: int = 1):
    if iters not in _CACHE:
        _CACHE[iters] = _Runner(build_nc(iters), N_CORES)
    return _CACHE[iters]


def prepare_inputs(x, W1, b1, W2, b2, Wo, bo, Wp, bp):
    """Host prep: transpose + bf16 hi/lo splits + weight folds -> per-core maps."""
    x = np.asarray(x, np.float32)
    W1 = np.asarray(W1, np.float32); b1 = np.asarray(b1, np.float32)
    W2 = np.asarray(W2, np.float32); b2 = np.asarray(b2, np.float32)
    Wo = np.asarray(Wo, np.float32); Wp = np.asarray(Wp, np.float32)
    bp = np.asarray(bp, np.float32)
    B = x.shape[0]
    assert B == BC * N_CORES, f"expected batch {BC * N_CORES}, got {B}"
    X = x.reshape(B, D)

    XT = np.ascontiguousarray(X.T)
    xt_hi = XT.astype(np.float16)
    xt_lo = (XT - xt_hi.astype(np.float32)).astype(ml_dtypes.bfloat16)

    wcat = np.concatenate([W1, Wo.T], axis=1)            # [D, 384]
    wcat_hi = wcat.astype(np.float16)
    w1_lo = (W1 - wcat_hi[:, :256].astype(np.float32)).astype(ml_dtypes.bfloat16)
    w2_hi = W2.astype(np.float16)                        # [256, 128]
    w2_lo = (W2 - w2_hi.astype(np.float32)).astype(ml_dtypes.bfloat16)
    wop = (Wo @ Wp).astype(np.float32).reshape(1, P)     # [1, 128]
    w1p = (W1.T @ Wp).astype(np.float32).reshape(256, 1)
    w2nb = (-W2 * b1.reshape(256, 1)).astype(np.float16)   # b1-row-scaled, negated
    # w1p-row-scaled, x16 so fp16 stationary stays in normal range (q is
    # scaled back by 1/16 when copied out of PSUM)
    w2q = (W2 * w1p * 16.0).astype(np.float16)
    # w2p16: [w2h c0 | w2h c1 | w2nb c0 | w2nb c1 | w2q c0 | w2q c1 | ones]
    w2p16 = np.concatenate(
        [w2_hi[:P], w2_hi[P:], w2nb[:P], w2nb[P:], w2q[:P], w2q[P:],
         np.ones((P, 1), np.float16)], axis=1)
    w2lo = np.concatenate([w2_lo[:P], w2_lo[P:]], axis=1)  # bf16
    cpack = np.zeros((P, 5), np.float32)
    cpack[:, 0:2] = b1.reshape(2, P).T
    cpack[:, 2:3] = b2.reshape(P, 1)
    cpack[0, 3] = float(bp.reshape(-1)[0])
    cpack[:, 4:5] = wop.T

    shared = {
        "wcat_hi": wcat_hi,
        "w1_lo": w1_lo,
        "w2p16": np.ascontiguousarray(w2p16),
        "w2lo": np.ascontiguousarray(w2lo),
        "cpack": cpack,
    }
    in_maps = []
    for c in range(N_CORES):
        cs = slice(c * BC, (c + 1) * BC)
        m = dict(shared)
        m["xt_hi"] = np.ascontiguousarray(xt_hi[:, cs])
        m["xt_lo"] = np.ascontiguousarray(xt_lo[:, cs])
        in_maps.append(m)
    return in_maps


def kernel(**inputs):
    in_maps = prepare_inputs(**inputs)
    runner = _get_runner(1)
    results = runner(in_maps)
    out = np.concatenate([results[c]["out"][0] for c in range(N_CORES)])
    return out.reshape(BC * N_CORES, 1).astype(np.float32)
